# revision 39
# baseline (speedup 1.0000x reference)
"""Multi-head attention Trainium2 kernel (8 NeuronCores).

Sharding: core c owns batch b = c//2 and heads h0 = (c%2)*6 .. h0+6 (tensor
parallel over heads x data parallel over batch). Each core computes its 6
heads' attention and a partial output projection; the host sums the two
partial projections per batch element and adds the output bias.

Per-core layout (all matmuls in float32r, fp32 PSUM accumulation):
  xt  [D=768, S=2048]   x[b] transposed on host, D on partitions
  Qt/Kt [384, 2048]     (h e) on partitions, computed as Wq^T @ x^T
  V   [S, 576]          keys on partitions; per head pair: [V_a|ones|V_b]
  scores^T [keys, q]    per 128-key chunk, via lhsT=Kt slice (K=64)
  exp on ScalarE PSUM->SBUF, fused 1/8 scale
  ctx^T += [V_h|ones]^T @ exp: 64 psum rows of unnormalized ctx +
                        64 rows of replicated softmax denominator
  normalize on VectorE, project with Wo chunks, DMA partial out.
"""
import sys

sys.path.insert(0, "/opt/trn_rl_repo")

from contextlib import ExitStack

import numpy as np

import concourse.bacc as bacc
import concourse.bass as bass
import concourse.mybir as mybir
import concourse.tile as tile
from concourse.bass_utils import run_bass_kernel_spmd

f32 = mybir.dt.float32
f32r = mybir.dt.float32r
AF = mybir.ActivationFunctionType
ALU = mybir.AluOpType

B, S, D = 4, 2048, 768
H, E = 12, 64
HL = 6              # heads per core
F = HL * E          # 384: local concat-head feature dim
ND = D // 128       # 6 contraction chunks over D
NF = F // 128       # 3 chunks over F
NK = S // 128       # 16 key chunks
QB = 512            # q block (matmul moving free dim)
NQB = S // QB       # 4
KG = 2              # key chunks per exp group
OW = 32            # ones block width (sums replicated OW rows)
VW = NF * (2 * E + OW)  # V tile width: 3 pairs x [V_a|ones|V_b]
NCORES = 8

_NC = None


def _build():
    nc = bacc.Bacc()
    xt_d = nc.declare_dram_parameter("xt", [D, S], f32r, isOutput=False)
    wq_d = nc.declare_dram_parameter("wq", [D, F], f32r, isOutput=False)
    wk_d = nc.declare_dram_parameter("wk", [D, F], f32r, isOutput=False)
    wv_d = nc.declare_dram_parameter("wv", [D, F], f32r, isOutput=False)
    wo_d = nc.declare_dram_parameter("wo", [F, D], f32r, isOutput=False)
    bq_d = nc.declare_dram_parameter("bq", [F, 1], f32, isOutput=False)
    bk_d = nc.declare_dram_parameter("bk", [F, 1], f32, isOutput=False)
    bv_d = nc.declare_dram_parameter("bv", [1, F], f32, isOutput=False)
    ones_d = nc.declare_dram_parameter("ones", [1, NF * OW], f32r, isOutput=False)
    y_d = nc.declare_dram_parameter("y", [S, D], f32, isOutput=True)

    with tile.TileContext(nc) as tc, ExitStack() as ctx:
        big = ctx.enter_context(tc.tile_pool(name="big", bufs=16))
        vpool = ctx.enter_context(tc.tile_pool(name="vpool", bufs=NK))
        wpool = ctx.enter_context(tc.tile_pool(name="wpool", bufs=9))
        wopool = ctx.enter_context(tc.tile_pool(name="wopool", bufs=3))
        epool = ctx.enter_context(tc.tile_pool(name="epool", bufs=2))
        opool = ctx.enter_context(tc.tile_pool(name="opool", bufs=3))
        npool = ctx.enter_context(tc.tile_pool(name="npool", bufs=1))
        cpool = ctx.enter_context(tc.tile_pool(name="cpool", bufs=1))
        psA = ctx.enter_context(tc.tile_pool(name="psA", bufs=4, space="PSUM"))
        pssc = ctx.enter_context(tc.tile_pool(name="pssc", bufs=2, space="PSUM"))

        # --- constant/bias tiles ---
        bq_sb = cpool.tile([128, NF], f32, name="bq_sb", tag="bq")
        nc.sync.dma_start(out=bq_sb, in_=bq_d.rearrange("(m p) o -> p m o", p=128))
        bk_sb = cpool.tile([128, NF], f32, name="bk_sb", tag="bk")
        nc.sync.dma_start(out=bk_sb, in_=bk_d.rearrange("(m p) o -> p m o", p=128))
        # bv broadcast across partitions via 0-stride DRAM read
        bv_bc = cpool.tile([128, F], f32, name="bv_bc", tag="bv")
        bv_src = bv_d[0:1, :]
        bv_ap = bass.AP(tensor=bv_src.tensor, offset=bv_src.offset,
                        ap=[[0, 128]] + list(bv_src.ap)[1:])
        nc.sync.dma_start(out=bv_bc, in_=bv_ap)

        # --- input/weight tiles: gpsimd DMA casts f32 -> f32r on the fly ---
        xt_t = []
        for kd in range(ND):
            t = big.tile([128, S], f32r, tag="big", name=f"xt{kd}")
            eng = nc.sync if kd % 2 == 0 else nc.scalar
            eng.dma_start(out=t, in_=xt_d[kd * 128:(kd + 1) * 128, :])
            xt_t.append(t)
        v_t = []
        for mk in range(NK):
            t = vpool.tile([128, VW], f32r, tag="v", name=f"v{mk}")
            t3o = t[:].rearrange("p (pair c) -> p pair c", c=2 * E + OW)
            src = xt_t[0][:, 0:NF * OW].rearrange("p (pair e) -> p pair e", e=OW)
            nc.vector.tensor_scalar(
                t3o[:, :, E:E + OW], src, 0.0, 1.0,
                op0=ALU.mult, op1=ALU.add)
            v_t.append(t)
        wq_t, wk_t, wv_t = [], [], []
        for nm, dd, lst in (("wv", wv_d, wv_t), ("wk", wk_d, wk_t), ("wq", wq_d, wq_t)):
            for kd in range(ND):
                t = wpool.tile([128, F], f32r, tag="w", name=f"{nm}{kd}")
                nc.sync.dma_start(out=t, in_=dd[kd * 128:(kd + 1) * 128, :])
                lst.append(t)
        wo_t = []
        for kf in range(NF):
            t = wopool.tile([128, D], f32r, tag="wo", name=f"wo{kf}")
            nc.sync.dma_start(out=t, in_=wo_d[kf * 128:(kf + 1) * 128, :])
            wo_t.append(t)

        # --- Qt (zero-padded per head, K=128 scores), Kt paired ---
        qt_t = [big.tile([128, S], f32r, tag="big", name=f"qt{h}") for h in range(2 * NF)]
        kt_t = [big.tile([128, S], f32r, tag="big", name=f"kt{m}") for m in range(NF)]
        for m in range(NF):
            nc.vector.tensor_scalar_mul(qt_t[2 * m][E:128, :], xt_t[0][E:128, :], 0.0)
            nc.vector.tensor_scalar_mul(qt_t[2 * m + 1][0:E, :], xt_t[0][0:E, :], 0.0)

        def emit_k_group(m, nq):
            ps = psA.tile([128, QB], f32, tag="bank", name=f"p1k_{m}_{nq}")
            for kd in range(ND):
                nc.tensor.matmul(
                    ps[:, :],
                    lhsT=wk_t[kd][:, m * 128:(m + 1) * 128],
                    rhs=xt_t[kd][:, nq * QB:(nq + 1) * QB],
                    start=(kd == 0), stop=(kd == ND - 1),
                )
            nc.vector.tensor_scalar_add(
                kt_t[m][:, nq * QB:(nq + 1) * QB], ps[:, :], bk_sb[:, m:m + 1])

        def emit_q_group(m, nq):
            sl = slice(nq * QB, (nq + 1) * QB)
            ps = psA.tile([128, QB], f32, tag="bank", name=f"p1q_{m}_{nq}")
            for kd in range(ND):
                nc.tensor.matmul(
                    ps[:, :],
                    lhsT=wq_t[kd][:, m * 128:(m + 1) * 128],
                    rhs=xt_t[kd][:, nq * QB:(nq + 1) * QB],
                    start=(kd == 0), stop=(kd == ND - 1),
                )
            nc.vector.tensor_scalar_add(
                qt_t[2 * m][0:E, sl], ps[0:E, :], bq_sb[0:E, m:m + 1])
            nc.vector.tensor_scalar_add(
                qt_t[2 * m + 1][E:128, sl], ps[E:128, :], bq_sb[E:128, m:m + 1])

        # --- V first: attention ctx needs all of it ---
        for mk in range(NK):
            ps = psA.tile([128, F], f32, tag="bank", name=f"p1v_{mk}", padded_shape=[128, QB])
            for kd in range(ND):
                nc.tensor.matmul(
                    ps[:, :],
                    lhsT=xt_t[kd][:, mk * 128:(mk + 1) * 128],
                    rhs=wv_t[kd][:, :],
                    start=(kd == 0), stop=(kd == ND - 1),
                )
            t = v_t[mk]
            ps3 = ps.rearrange("p (pair hh e) -> p pair hh e", hh=2, e=E)
            bv3 = bv_bc.rearrange("p (pair hh e) -> p pair hh e", hh=2, e=E)
            t3 = t[:].rearrange("p (pair c) -> p pair c", c=2 * E + OW)
            nc.vector.tensor_tensor(
                t3[:, :, 0:E], ps3[:, :, 0, :], bv3[:, :, 0, :], op=ALU.add)
            nc.vector.tensor_tensor(
                t3[:, :, E + OW:2 * E + OW], ps3[:, :, 1, :], bv3[:, :, 1, :], op=ALU.add)

        # pair-0 K/Q projections up front; pairs 1-2 are emitted as fillers
        # inside head-pair-0's attention units (the PE has slack there while
        # ScalarE grinds through exp).
        for m in range(NF):
            for nq in range(NQB):
                emit_k_group(m, nq)
        for m in range(NF):
            for nq in range(NQB):
                emit_q_group(m, nq)
        qk_fill = []

        # --- output projection, per head pair, DMA-accumulated into y ---
        ctxt_t = [big.tile([128, S], f32r, tag="big", name=f"ctxt{m}") for m in range(NF)]


        def emit_proj(mq):
            # output projection for query rows mq*128..+128, accumulating
            # all head pairs in PSUM
            osb = opool.tile([128, D], f32, tag="o", name=f"ot{mq}")
            for piece, (c0, c1) in enumerate(((0, 512), (512, D))):
                ps = psA.tile([128, c1 - c0], f32, tag="bank",
                              name=f"o{piece}_{mq}", padded_shape=[128, QB])
                for kf in range(NF):
                    nc.tensor.matmul(
                        ps[:, :], lhsT=ctxt_t[kf][:, mq * 128:(mq + 1) * 128],
                        rhs=wo_t[kf][:, c0:c1],
                        start=(kf == 0), stop=(kf == NF - 1))
                nc.vector.tensor_copy(osb[:, c0:c1], ps[:, :])
            nc.sync.dma_start(out=y_d[mq * 128:(mq + 1) * 128, :], in_=osb[:])

        # --- attention ---
        for hp in range(NF):          # head pair (partitions 0:64 / 64:128)
            for nq in range(NQB):
                cps = [
                    psA.tile([128, QB], f32, tag="bank", name=f"c{hp}_{nq}_{hh}")
                    for hh in range(2)
                ]
                for _ in range(2):
                    if qk_fill:
                        m, fnq, w = qk_fill.pop(0)
                        (emit_k_group if w == "k" else emit_q_group)(m, fnq)
                def normalize(hh):
                    # head a: ctx rows 0:64, sums 64:64+OW; head b: sums
                    # rows 0:OW, ctx OW:OW+64.
                    c0 = 0 if hh == 0 else OW
                    s0 = E if hh == 0 else 0
                    r = npool.tile([OW, QB], f32, tag="n",
                                   name=f"n{hp}_{nq}_{hh}", bufs=2)
                    nc.vector.reciprocal(r[:], cps[hh][s0:s0 + OW, :])
                    for half in range(E // OW):
                        nc.vector.tensor_tensor(
                            ctxt_t[hp][hh * E + half * OW:hh * E + (half + 1) * OW,
                                       nq * QB:(nq + 1) * QB],
                            cps[hh][c0 + half * OW:c0 + (half + 1) * OW, :],
                            r[:], op=ALU.mult,
                        )

                for g in range(NK // KG):
                    for hh in range(2):
                        sps = pssc.tile([128, KG * QB], f32, tag="sc",
                                        name=f"s{hp}_{nq}_{g}_{hh}")
                        for j in range(KG):
                            mk = g * KG + j
                            nc.tensor.matmul(
                                sps[:, j * QB:(j + 1) * QB],
                                lhsT=kt_t[hp][:, mk * 128:(mk + 1) * 128],
                                rhs=qt_t[2 * hp + hh][:, nq * QB:(nq + 1) * QB],
                                start=True, stop=True,
                            )
                        esb = epool.tile([128, KG * QB], f32r, tag="e",
                                         name=f"e{hp}_{nq}_{g}_{hh}")
                        nc.scalar.activation(esb[:], sps[:], AF.Exp, scale=0.125)
                        if g == 3 and hh == 0 and qk_fill and len(qk_fill) % 2 == 0:
                            m, fnq, w = qk_fill.pop(0)
                            (emit_k_group if w == "k" else emit_q_group)(m, fnq)
                        for j in range(KG):
                            mk = g * KG + j
                            base = hp * (2 * E + OW) + hh * E
                            nc.tensor.matmul(
                                cps[hh][0:E + OW, :],
                                lhsT=v_t[mk][:, base:base + E + OW],
                                rhs=esb[:, j * QB:(j + 1) * QB],
                                start=(g == 0 and j == 0),
                                stop=(g == NK // KG - 1 and j == KG - 1),
                            )
                        if g == NK // KG - 1:
                            normalize(hh)
                            if hh == 1 and hp == NF - 1 and nq < NQB - 1:
                                for i in range(4):
                                    emit_proj(nq * 4 + i)
        for mq in range(4 * (NQB - 1), NK):
            emit_proj(mq)
    nc.compile()
    return nc


def _get_nc():
    global _NC
    if _NC is None:
        _NC = _build()
    return _NC


def kernel(x, Wq, bq, Wk, bk, Wv, bv, Wo, bo, _trace=False):
    x = np.asarray(x, dtype=np.float32)
    Wq = np.asarray(Wq, dtype=np.float32)
    bq = np.asarray(bq, dtype=np.float32)
    Wk = np.asarray(Wk, dtype=np.float32)
    bk = np.asarray(bk, dtype=np.float32)
    Wv = np.asarray(Wv, dtype=np.float32)
    bv = np.asarray(bv, dtype=np.float32)
    Wo = np.asarray(Wo, dtype=np.float32)
    bo = np.asarray(bo, dtype=np.float32)

    nc = _get_nc()
    in_maps = []
    for c in range(NCORES):
        b = c // 2
        h0 = (c % 2) * HL
        in_maps.append({
            "xt": np.ascontiguousarray(x[b].T),
            "wq": np.ascontiguousarray(Wq[h0:h0 + HL].transpose(1, 0, 2).reshape(D, F)),
            "wk": np.ascontiguousarray(Wk[h0:h0 + HL].transpose(1, 0, 2).reshape(D, F)),
            "wv": np.ascontiguousarray(Wv[h0:h0 + HL].transpose(1, 0, 2).reshape(D, F)),
            "wo": np.ascontiguousarray(Wo[h0 * E:(h0 + HL) * E]),
            "bq": np.ascontiguousarray(bq[h0:h0 + HL].reshape(F, 1)),
            "bk": np.ascontiguousarray(bk[h0:h0 + HL].reshape(F, 1)),
            "bv": np.ascontiguousarray(bv[h0:h0 + HL].reshape(1, F)),
            "ones": np.ones((1, NF * OW), np.float32),
        })
    res = run_bass_kernel_spmd(nc, in_maps, list(range(NCORES)), trace=_trace)
    out = np.empty((B, S, D), np.float32)
    for b in range(B):
        out[b] = res.results[2 * b]["y"] + res.results[2 * b + 1]["y"] + bo[None, :]
    if _trace:
        kernel.last_exec_time_ns = res.exec_time_ns
        kernel.last_results = res
    return out
